# revision 9
# baseline (speedup 1.0000x reference)
"""FAGCNConv Trainium2 kernel (8 NeuronCores, destination-sharded edges).

Algorithm (matches reference up to fp rounding):
    s1 = x @ w1            (per node, all nodes)
    s2b = x @ w2 + b       (per node, local destination slice)
    p_e = exp(tanh(s1[row_e] + s2b[col_e]))            # scores bounded by tanh,
                                                       # so softmax max-shift is unnecessary
    acc[v]    = sum_{e: col=v} p_e * x[row_e]          # via one-hot matmul in PSUM
    segsum[v] = sum_{e: col=v} p_e                     # extra ones-channel of same matmul
    out[v] = (1-EPS) * acc[v]/segsum[v] + EPS * x[v]

Sharding: core c owns destination nodes [c*6250, (c+1)*6250). Host sorts edges by
destination block (128 dst nodes per block), pads each block to a uniform number
of 128-edge tiles. Per tile, a [128e x 128d] one-hot scatter matrix is built on
DVE (iota == colrel) and accumulated on the tensor engine into a [128d, 129]
PSUM accumulator. Gathers of x[row], s1[row], s2b[col] use batched indirect DMA.
"""

import os
import sys

sys.path.insert(0, "/opt/trn_rl_repo")

import numpy as np

N_NODES = 50000
N_EDGES = 800000
C = 128
EPS = 0.1
NCORES = 8
NLOC = N_NODES // NCORES          # 6250 destination nodes per core
NBLK = (NLOC + 127) // 128        # 49 blocks (48 full, last has 106 dst)
LAST_BLK_N = NLOC - 128 * (NBLK - 1)   # 106
P = 128
DUMMY_COLREL = 200.0


def _prep_shards(edge_index: np.ndarray):
    """Sort/pad edges per core. Returns (TB, per-core dict of tables)."""
    row_g = edge_index[0].astype(np.int64)
    col_g = edge_index[1].astype(np.int64)
    core = col_g // NLOC

    per_core = []
    max_blk_edges = 0
    for c in range(NCORES):
        m = core == c
        r = row_g[m]
        cl = col_g[m] - c * NLOC
        blk = cl // P
        counts = np.bincount(blk, minlength=NBLK)
        max_blk_edges = max(max_blk_edges, int(counts.max()))
        per_core.append((r, cl, blk, counts))

    TB = (max_blk_edges + P - 1) // P  # tiles per block (uniform)

    shards = []
    for c in range(NCORES):
        r, cl, blk, counts = per_core[c]
        T = NBLK * TB
        idxr = np.zeros(T * P, dtype=np.int32)
        colrel = np.full(T * P, DUMMY_COLREL, dtype=np.float32)
        colloc = np.zeros(T * P, dtype=np.int32)
        order = np.argsort(blk, kind="stable")
        starts = np.zeros(NBLK, dtype=np.int64)
        starts[1:] = np.cumsum(counts)[:-1]
        # position of edge within its block
        pos_in_blk = np.arange(len(order)) - starts[blk[order]]
        slot = blk[order] * (TB * P) + pos_in_blk
        idxr[slot] = r[order]
        colrel[slot] = (cl[order] - blk[order] * P).astype(np.float32)
        colloc[slot] = cl[order]
        shards.append(
            dict(
                idxr_T=np.ascontiguousarray(idxr.reshape(T, P).T),      # [128, T]
                colrel_T=np.ascontiguousarray(colrel.reshape(T, P).T),  # [128, T]
                colloc_T=np.ascontiguousarray(colloc.reshape(T, P).T),  # [128, T]
            )
        )
    return TB, shards


def _build_nc(TB):
    import concourse.bacc as bacc
    import concourse.bass as bass
    import concourse.mybir as mybir
    from concourse.tile import TileContext

    f32 = mybir.dt.float32
    i32 = mybir.dt.int32
    T = NBLK * TB

    nc = bacc.Bacc("TRN2", target_bir_lowering=False)

    x_d = nc.dram_tensor("x", [N_NODES, C], f32, kind="ExternalInput")
    xloc_d = nc.dram_tensor("xloc", [NLOC, C], f32, kind="ExternalInput")
    idxr_d = nc.dram_tensor("idxr", [P, T], i32, kind="ExternalInput")
    colrel_d = nc.dram_tensor("colrel", [P, T], f32, kind="ExternalInput")
    colloc_d = nc.dram_tensor("colloc", [P, T], i32, kind="ExternalInput")
    gw_d = nc.dram_tensor("gate_w", [2 * C, 1], f32, kind="ExternalInput")
    gb_d = nc.dram_tensor("gate_b", [1], f32, kind="ExternalInput")
    ident_d = nc.dram_tensor("ident", [P, P], f32, kind="ExternalInput")
    iota_d = nc.dram_tensor("iotaf", [P, P], f32, kind="ExternalInput")
    out_d = nc.dram_tensor("out", [NLOC, C], f32, kind="ExternalOutput")

    s1_d = nc.dram_tensor("s1_scratch", [N_NODES, 1], f32)
    s2b_d = nc.dram_tensor("s2b_scratch", [NLOC, 1], f32)

    NT_ALL = (N_NODES + P - 1) // P   # 391 (last 80)
    NT_LOC = (NLOC + P - 1) // P      # 49  (last 106)

    with TileContext(nc) as tc:
        with (
            tc.tile_pool(name="const", bufs=1) as cpool,
            tc.tile_pool(name="phA", bufs=4) as apool,
            tc.tile_pool(name="phA_ps", bufs=2, space="PSUM") as apsum,
            tc.tile_pool(name="blk", bufs=2) as bpool,
            tc.tile_pool(name="small", bufs=8) as spool,
            tc.tile_pool(name="edge", bufs=4) as epool,
            tc.tile_pool(name="acc_ps", bufs=2, space="PSUM") as bpsum,
        ):
            ident = cpool.tile([P, P], f32)
            nc.sync.dma_start(ident[:], ident_d[:])
            iotaf = cpool.tile([P, P], f32)
            nc.sync.dma_start(iotaf[:], iota_d[:])
            w1 = cpool.tile([P, 1], f32)
            nc.sync.dma_start(w1[:], gw_d[0:C, 0:1])
            w2 = cpool.tile([P, 1], f32)
            nc.sync.dma_start(w2[:], gw_d[C : 2 * C, 0:1])
            btile = cpool.tile([1, 1], f32)
            nc.sync.dma_start(btile[:], gb_d[:, None])

            # ---- Phase A: s1[v] = x[v] @ w1 for all nodes ----
            for nt in range(NT_ALL):
                a = nt * P
                nr = min(P, N_NODES - a)
                xt = apool.tile([P, C], f32, tag="xt")
                nc.sync.dma_start(xt[:nr, :], x_d[a : a + nr, :])
                xT_ps = apsum.tile([P, P], f32, tag="xT")
                nc.tensor.transpose(
                    out=xT_ps[:, :nr], in_=xt[:nr, :], identity=ident[:nr, :nr]
                )
                xT = apool.tile([P, P], f32, tag="xTs")
                nc.vector.tensor_copy(xT[:, :nr], xT_ps[:, :nr])
                sT_ps = apsum.tile([1, P], f32, tag="sT")
                nc.tensor.matmul(
                    out=sT_ps[:, :nr], lhsT=w1[:], rhs=xT[:, :nr], start=True, stop=True
                )
                sT = apool.tile([1, P], f32, tag="sTs")
                nc.vector.tensor_copy(sT[:, :nr], sT_ps[:, :nr])
                nc.sync.dma_start(s1_d[a : a + nr, 0:1], sT[0:1, :nr])

            # ---- Phase A2: s2b[v] = xloc[v] @ w2 + b for local nodes ----
            for nt in range(NT_LOC):
                a = nt * P
                nr = min(P, NLOC - a)
                xt = apool.tile([P, C], f32, tag="xt")
                nc.sync.dma_start(xt[:nr, :], xloc_d[a : a + nr, :])
                xT_ps = apsum.tile([P, P], f32, tag="xT")
                nc.tensor.transpose(
                    out=xT_ps[:, :nr], in_=xt[:nr, :], identity=ident[:nr, :nr]
                )
                xT = apool.tile([P, P], f32, tag="xTs")
                nc.vector.tensor_copy(xT[:, :nr], xT_ps[:, :nr])
                sT_ps = apsum.tile([1, P], f32, tag="sT")
                nc.tensor.matmul(
                    out=sT_ps[:, :nr], lhsT=w2[:], rhs=xT[:, :nr], start=True, stop=True
                )
                sT = apool.tile([1, P], f32, tag="sTs")
                nc.scalar.activation(
                    sT[:, :nr],
                    sT_ps[:, :nr],
                    mybir.ActivationFunctionType.Identity,
                    bias=btile[:],
                    scale=1.0,
                )
                nc.sync.dma_start(s2b_d[a : a + nr, 0:1], sT[0:1, :nr])

            # ---- Phase B: edge gather + gate + one-hot scatter matmul ----
            for b in range(NBLK):
                t0 = b * TB
                nd = min(P, NLOC - b * P)  # dst nodes in this block

                idxr_blk = epool.tile([P, TB], i32, tag="idxr")
                nc.sync.dma_start(idxr_blk[:], idxr_d[:, t0 : t0 + TB])
                colrel_blk = epool.tile([P, TB], f32, tag="colrel")
                nc.sync.dma_start(colrel_blk[:], colrel_d[:, t0 : t0 + TB])
                colloc_blk = epool.tile([P, TB], i32, tag="colloc")
                nc.sync.dma_start(colloc_blk[:], colloc_d[:, t0 : t0 + TB])

                # HW indirect DMA honors exactly one offset per partition, so
                # gathers are per 128-edge tile.
                Y_blk = bpool.tile([P, TB * C], f32, tag="Y")
                sr_blk = spool.tile([P, TB], f32, tag="sr")
                sc_blk = spool.tile([P, TB], f32, tag="sc")
                for t in range(TB):
                    nc.gpsimd.indirect_dma_start(
                        out=Y_blk[:, t * C : (t + 1) * C],
                        out_offset=None,
                        in_=x_d[:],
                        in_offset=bass.IndirectOffsetOnAxis(
                            ap=idxr_blk[:, t : t + 1], axis=0
                        ),
                    )
                    nc.gpsimd.indirect_dma_start(
                        out=sr_blk[:, t : t + 1],
                        out_offset=None,
                        in_=s1_d[:],
                        in_offset=bass.IndirectOffsetOnAxis(
                            ap=idxr_blk[:, t : t + 1], axis=0
                        ),
                    )
                    nc.gpsimd.indirect_dma_start(
                        out=sc_blk[:, t : t + 1],
                        out_offset=None,
                        in_=s2b_d[:],
                        in_offset=bass.IndirectOffsetOnAxis(
                            ap=colloc_blk[:, t : t + 1], axis=0
                        ),
                    )

                acc = bpsum.tile([P, C + 1], f32, tag="acc")
                for t in range(TB):
                    u = spool.tile([P, 1], f32, tag="u")
                    nc.scalar.activation(
                        u[:],
                        sr_blk[:, t : t + 1],
                        mybir.ActivationFunctionType.Tanh,
                        bias=sc_blk[:, t : t + 1],
                        scale=1.0,
                    )
                    pv = spool.tile([P, 1], f32, tag="pv")
                    nc.scalar.activation(
                        pv[:], u[:], mybir.ActivationFunctionType.Exp
                    )
                    onehot = epool.tile([P, P], f32, tag="onehot")
                    nc.vector.tensor_scalar(
                        onehot[:],
                        iotaf[:],
                        colrel_blk[:, t : t + 1],
                        None,
                        op0=mybir.AluOpType.is_equal,
                    )
                    Z = epool.tile([P, C + 1], f32, tag="Z")
                    nc.vector.tensor_scalar(
                        Z[:, :C],
                        Y_blk[:, t * C : (t + 1) * C],
                        pv[:],
                        None,
                        op0=mybir.AluOpType.mult,
                    )
                    nc.vector.tensor_copy(Z[:, C : C + 1], pv[:])
                    nc.tensor.matmul(
                        out=acc[:],
                        lhsT=onehot[:],
                        rhs=Z[:],
                        start=(t == 0),
                        stop=(t == TB - 1),
                    )

                segsum = spool.tile([P, 1], f32, tag="segsum")
                nc.vector.tensor_scalar(
                    segsum[:], acc[:, C : C + 1], 1e-30, None,
                    op0=mybir.AluOpType.add,
                )
                inv = spool.tile([P, 1], f32, tag="inv")
                nc.vector.reciprocal(inv[:], segsum[:])
                inv9 = spool.tile([P, 1], f32, tag="inv9")
                nc.scalar.mul(inv9[:], inv[:], 1.0 - EPS)

                xblk = bpool.tile([P, C], f32, tag="xblk")
                nc.sync.dma_start(xblk[:nd, :], xloc_d[b * P : b * P + nd, :])
                o1 = bpool.tile([P, C], f32, tag="o1")
                nc.vector.tensor_scalar(
                    o1[:], acc[:, :C], inv9[:], None, op0=mybir.AluOpType.mult
                )
                oblk = bpool.tile([P, C], f32, tag="oblk")
                nc.vector.scalar_tensor_tensor(
                    oblk[:nd, :],
                    xblk[:nd, :],
                    EPS,
                    o1[:nd, :],
                    op0=mybir.AluOpType.mult,
                    op1=mybir.AluOpType.add,
                )
                nc.sync.dma_start(out_d[b * P : b * P + nd, :], oblk[:nd, :])

    nc.finalize()
    return nc


_CACHE = {}


def _get_nc(TB):
    if TB not in _CACHE:
        _CACHE[TB] = _build_nc(TB)
    return _CACHE[TB]


def kernel(x, edge_index, gate_w, gate_b):
    from concourse.bass_utils import run_bass_kernel_spmd

    x = np.asarray(x, dtype=np.float32)
    edge_index = np.asarray(edge_index, dtype=np.int32)
    gate_w = np.asarray(gate_w, dtype=np.float32)
    gate_b = np.asarray(gate_b, dtype=np.float32)

    TB, shards = _prep_shards(edge_index)
    nc = _get_nc(TB)

    ident = np.eye(P, dtype=np.float32)
    iotaf = np.broadcast_to(
        np.arange(P, dtype=np.float32)[None, :], (P, P)
    ).copy()

    in_maps = []
    for c in range(NCORES):
        in_maps.append(
            {
                "x": x,
                "xloc": np.ascontiguousarray(x[c * NLOC : (c + 1) * NLOC]),
                "idxr": shards[c]["idxr_T"],
                "colrel": shards[c]["colrel_T"],
                "colloc": shards[c]["colloc_T"],
                "gate_w": gate_w,
                "gate_b": gate_b,
                "ident": ident,
                "iotaf": iotaf,
            }
        )

    res = run_bass_kernel_spmd(nc, in_maps, core_ids=list(range(NCORES)))
    out = np.concatenate([res.results[c]["out"] for c in range(NCORES)], axis=0)
    return out


def _make_in_maps(x, edge_index, gate_w, gate_b):
    TB, shards = _prep_shards(edge_index)
    ident = np.eye(P, dtype=np.float32)
    iotaf = np.broadcast_to(np.arange(P, dtype=np.float32)[None, :], (P, P)).copy()
    in_maps = []
    for c in range(NCORES):
        in_maps.append(
            {
                "x": x,
                "xloc": np.ascontiguousarray(x[c * NLOC : (c + 1) * NLOC]),
                "idxr": shards[c]["idxr_T"],
                "colrel": shards[c]["colrel_T"],
                "colloc": shards[c]["colloc_T"],
                "gate_w": gate_w,
                "gate_b": gate_b,
                "ident": ident,
                "iotaf": iotaf,
            }
        )
    return TB, in_maps


def time_kernel(inputs, iters=12, iters_lo=2, reps=5):
    """Estimate per-execution HW time by chaining NEFF executions in one jit.

    Chained call k feeds call k-1's outputs in as the (unused-but-dependency-
    creating) output operands, preventing CSE/reordering. Per-exec time =
    (T(iters) - T(iters_lo)) / (iters - iters_lo), minimum over reps.
    """
    import time as _time

    import jax
    import concourse.mybir as mybir
    from concourse import bass2jax as b2j

    x = np.asarray(inputs["x"], dtype=np.float32)
    edge_index = np.asarray(inputs["edge_index"], dtype=np.int32)
    gate_w = np.asarray(inputs["gate_w"], dtype=np.float32)
    gate_b = np.asarray(inputs["gate_b"], dtype=np.float32)

    TB, in_maps = _make_in_maps(x, edge_index, gate_w, gate_b)
    nc = _get_nc(TB)
    b2j.install_neuronx_cc_hook()

    partition_name = nc.partition_id_tensor.name if nc.partition_id_tensor else None
    in_names, out_names, out_avals, zero_outs = [], [], [], []
    for alloc in nc.m.functions[0].allocations:
        if not isinstance(alloc, mybir.MemoryLocationSet):
            continue
        name = alloc.memorylocations[0].name
        if alloc.kind == "ExternalInput":
            if name != partition_name:
                in_names.append(name)
        elif alloc.kind == "ExternalOutput":
            shape = tuple(alloc.tensor_shape)
            dtype = mybir.dt.np(alloc.dtype)
            out_names.append(name)
            out_avals.append(jax.core.ShapedArray(shape, dtype))
            zero_outs.append(np.zeros(shape, dtype))
    n_params = len(in_names)
    all_in_names = in_names + out_names

    def _make_chain(n_iter):
        del n_iter

        def _chain(*args):
            operands = list(args)
            if partition_name is not None:
                operands.append(b2j.partition_id_tensor())
            return tuple(
                b2j._bass_exec_p.bind(
                    *operands,
                    out_avals=tuple(out_avals),
                    in_names=tuple(
                        all_in_names + ([partition_name] if partition_name else [])
                    ),
                    out_names=tuple(out_names),
                    lowering_input_output_aliases=(),
                    sim_require_finite=True,
                    sim_require_nnan=True,
                    nc=nc,
                )
            )

        devices = jax.devices()[:NCORES]
        mesh = b2j.Mesh(np.asarray(devices), ("core",))
        in_specs = (b2j.PartitionSpec("core"),) * (n_params + len(out_names))
        out_specs = (b2j.PartitionSpec("core"),) * len(out_names)
        return jax.jit(
            b2j.shard_map(
                _chain, mesh=mesh, in_specs=in_specs, out_specs=out_specs,
                check_rep=False,
            ),
            keep_unused=True,
        ), mesh

    per_core = [[np.asarray(m[name]) for name in in_names] for m in in_maps]
    concat_in = [
        np.concatenate([per_core[c][i] for c in range(NCORES)], axis=0)
        for i in range(n_params)
    ]
    concat_zeros = [
        np.zeros((NCORES * z.shape[0], *z.shape[1:]), z.dtype) for z in zero_outs
    ]

    fn, mesh = _make_chain(1)

    from jax.sharding import NamedSharding

    sh = NamedSharding(mesh, b2j.PartitionSpec("core"))
    dev_in = [jax.device_put(a, sh) for a in concat_in]
    dev_zero = [jax.device_put(a, sh) for a in concat_zeros]

    # warmup (compile + first exec)
    jax.block_until_ready(fn(*dev_in, *dev_zero))
    jax.block_until_ready(fn(*dev_in, *dev_zero))

    M_hi, M_lo = iters, iters_lo
    best = None
    for _ in range(reps):
        t0 = _time.perf_counter()
        rs = [fn(*dev_in, *dev_zero) for _ in range(M_hi)]
        jax.block_until_ready(rs)
        t_hi = _time.perf_counter() - t0
        del rs
        t0 = _time.perf_counter()
        rs = [fn(*dev_in, *dev_zero) for _ in range(M_lo)]
        jax.block_until_ready(rs)
        t_lo = _time.perf_counter() - t0
        del rs
        per_exec = (t_hi - t_lo) / (M_hi - M_lo)
        print(
            f"  t({M_hi})={t_hi*1e3:.2f}ms t({M_lo})={t_lo*1e3:.2f}ms "
            f"per_exec={per_exec*1e6:.1f}us"
        )
        if best is None or per_exec < best:
            best = per_exec
    return best * 1e9


# revision 15
# speedup vs baseline: 3.3204x; 3.3204x over previous
"""FAGCNConv Trainium2 kernel (8 NeuronCores, destination-sharded edges). v2

Algorithm (matches reference up to fp rounding):
    s2b = x @ w2 + b                      (per destination node, local slice)
    sr_e = x[row_e] @ w1                  (per edge, fused DVE mul-reduce on gathered rows)
    sc_e = s2b[col_e]                     (per edge, fused one-hot dot vs broadcast s2 block)
    p_e  = exp(tanh(sr_e + sc_e))         (tanh bounds scores, so softmax max-shift is unneeded)
    acc[v], segsum[v] = sum_{e->v} p_e * [x[row_e] | 1]   (one-hot matmul into PSUM)
    out[v] = (1-EPS) * acc[v]/segsum[v] + EPS * x[v]

Sharding: core c owns destinations [c*6250, (c+1)*6250), 49 blocks of 128 dst.
Host sorts edges by (block, row>=32768) and pads each block's lo/hi sections to
uniform tile counts so the SPMD program is identical across cores. x rows are
fetched with dma_gather (int16 indices; the 50000-row table is addressed as a
low half and a +32768-offset high half).
"""

import os
import sys

sys.path.insert(0, "/opt/trn_rl_repo")

import numpy as np

N_NODES = 50000
C = 128
EPS = 0.1
NCORES = 8
NLOC = N_NODES // NCORES          # 6250
NBLK = (NLOC + 127) // 128        # 49 (48 full, last has 106 dst)
P = 128
HALF = 32768                      # int16 index limit for dma_gather
DUMMY_COLREL = 200.0
A2_CHUNK = 512


def _wrap_idx16(lst):
    """dma_gather index layout: [128, N/16] int16; idx i at [i%16, i//16],
    replicated across the 8 groups of 16 partitions."""
    n = len(lst)
    assert n % 128 == 0
    a16 = np.zeros((16, n // 16), dtype=np.int16)
    a16[np.arange(n) % 16, np.arange(n) // 16] = lst
    return np.tile(a16, (8, 1))


def _prep_shards(edge_index: np.ndarray):
    row_g = edge_index[0].astype(np.int64)
    col_g = edge_index[1].astype(np.int64)
    core_of = col_g // NLOC

    per_core = []
    max_lo = 0
    max_hi = 0
    for c in range(NCORES):
        m = core_of == c
        r = row_g[m]
        cl = col_g[m] - c * NLOC
        blk = cl // P
        hi = (r >= HALF).astype(np.int64)
        key = blk * 2 + hi
        counts = np.bincount(key, minlength=NBLK * 2)
        max_lo = max(max_lo, int(counts[0::2].max()))
        max_hi = max(max_hi, int(counts[1::2].max()))
        per_core.append((r, cl, blk, hi, key, counts))

    TBL = (max_lo + P - 1) // P
    TBH = (max_hi + P - 1) // P
    TB = TBL + TBH
    TILES = NBLK * TB

    shards = []
    for c in range(NCORES):
        r, cl, blk, hi, key, counts = per_core[c]
        idx_slot = np.zeros(TILES * P, dtype=np.int64)
        colrel_slot = np.full(TILES * P, DUMMY_COLREL, dtype=np.float32)

        order = np.argsort(key, kind="stable")
        starts = np.zeros(NBLK * 2, dtype=np.int64)
        starts[1:] = np.cumsum(counts)[:-1]
        pos_in_sec = np.arange(len(order)) - starts[key[order]]
        ro, clo, blko, hio = r[order], cl[order], blk[order], hi[order]
        sec_base = blko * (TB * P) + hio * (TBL * P)
        slot = sec_base + pos_in_sec
        idx_slot[slot] = ro - hio * HALF
        colrel_slot[slot] = (clo - blko * P).astype(np.float32)

        # per-section wrapped int16 index arrays, concatenated along columns
        idx16_cols = []
        for b in range(NBLK):
            base = b * TB * P
            idx16_cols.append(_wrap_idx16(idx_slot[base : base + TBL * P]))
            idx16_cols.append(
                _wrap_idx16(idx_slot[base + TBL * P : base + TB * P])
            )
        idx16 = np.concatenate(idx16_cols, axis=1)  # [128, TILES*8]
        colrel_T = np.ascontiguousarray(
            colrel_slot.reshape(TILES, P).T
        )  # [128, TILES]
        shards.append(dict(idx16=idx16, colrel_T=colrel_T))
    return TBL, TBH, shards


def _build_nc(TBL, TBH):
    import concourse.bacc as bacc
    import concourse.bass as bass
    import concourse.mybir as mybir
    from concourse.tile import TileContext

    f32 = mybir.dt.float32
    i16 = mybir.dt.int16
    TB = TBL + TBH
    TILES = NBLK * TB
    NLOC_PAD = NBLK * P  # 6272

    nc = bacc.Bacc("TRN2", target_bir_lowering=False)

    x_d = nc.dram_tensor("x", [N_NODES, C], f32, kind="ExternalInput")
    xhi_d = nc.dram_tensor("xhi", [N_NODES - HALF, C], f32, kind="ExternalInput")
    xloc_d = nc.dram_tensor("xloc", [NLOC, C], f32, kind="ExternalInput")
    xlocT_d = nc.dram_tensor("xlocT", [P, NLOC], f32, kind="ExternalInput")
    idx16_d = nc.dram_tensor("idx16", [P, TILES * 8], i16, kind="ExternalInput")
    colrel_d = nc.dram_tensor("colrel", [P, TILES], f32, kind="ExternalInput")
    gw_d = nc.dram_tensor("gate_w", [2 * C, 1], f32, kind="ExternalInput")
    gb_d = nc.dram_tensor("gate_b", [1], f32, kind="ExternalInput")
    iota_d = nc.dram_tensor("iotaf", [P, P], f32, kind="ExternalInput")
    w1b_d = nc.dram_tensor("w1b", [P, P], f32, kind="ExternalInput")
    out_d = nc.dram_tensor("out", [NLOC, C], f32, kind="ExternalOutput")

    s2b_d = nc.dram_tensor("s2b_scratch", [1, NLOC_PAD], f32)

    with TileContext(nc) as tc:
        with (
            tc.tile_pool(name="const", bufs=1) as cpool,
            tc.tile_pool(name="phA", bufs=3) as apool,
            tc.tile_pool(name="phA_ps", bufs=2, space="PSUM") as apsum,
            tc.tile_pool(name="ybuf", bufs=2) as ypool,
            tc.tile_pool(name="blk", bufs=3) as bpool,
            tc.tile_pool(name="small", bufs=4) as spool,
            tc.tile_pool(name="oh", bufs=22) as ohpool,
            tc.tile_pool(name="acc_ps", bufs=2, space="PSUM") as bpsum,
        ):
            iotaf = cpool.tile([P, P], f32)
            nc.sync.dma_start(iotaf[:], iota_d[:])
            w1b = cpool.tile([P, P], f32)
            nc.sync.dma_start(w1b[:], w1b_d[:])
            w2 = cpool.tile([P, 1], f32)
            nc.sync.dma_start(w2[:], gw_d[C : 2 * C, 0:1])
            btile = cpool.tile([1, 1], f32)
            nc.sync.dma_start(btile[:], gb_d[:, None])
            ones_col = cpool.tile([P, 1], f32)
            nc.vector.memset(ones_col[:], 1.0)
            zpad = cpool.tile([1, NLOC_PAD - NLOC], f32)
            nc.vector.memset(zpad[:], 0.0)
            nc.sync.dma_start(s2b_d[0:1, NLOC:NLOC_PAD], zpad[:])

            # ---- Phase A: s2b[v] = xloc[v] @ w2 + b (local nodes) ----
            nck = (NLOC + A2_CHUNK - 1) // A2_CHUNK
            for k in range(nck):
                a = k * A2_CHUNK
                n = min(A2_CHUNK, NLOC - a)
                xck = apool.tile([P, A2_CHUNK], f32, tag="xck")
                nc.sync.dma_start(xck[:, :n], xlocT_d[:, a : a + n])
                ps = apsum.tile([1, A2_CHUNK], f32, tag="s2ps")
                nc.tensor.matmul(
                    out=ps[:, :n], lhsT=w2[:], rhs=xck[:, :n], start=True, stop=True
                )
                s2sb = apool.tile([1, A2_CHUNK], f32, tag="s2sb")
                nc.scalar.activation(
                    s2sb[:, :n],
                    ps[:, :n],
                    mybir.ActivationFunctionType.Identity,
                    bias=btile[:],
                    scale=1.0,
                )
                nc.sync.dma_start(s2b_d[0:1, a : a + n], s2sb[:, :n])

            # ---- Phase B ----
            nblk_run = int(os.environ.get("KERNEL_NBLK", NBLK))
            skips = set(os.environ.get("KERNEL_SKIP", "").split(","))
            for b in range(nblk_run):
                nd = min(P, NLOC - b * P)
                t0 = b * TB

                colrel_blk = spool.tile([P, TB], f32, tag="colrel")
                nc.sync.dma_start(colrel_blk[:], colrel_d[:, t0 : t0 + TB])
                idxlo = spool.tile([P, TBL * 8], i16, tag="idxlo")
                nc.sync.dma_start(
                    idxlo[:], idx16_d[:, t0 * 8 : t0 * 8 + TBL * 8]
                )
                idxhi = spool.tile([P, TBH * 8], i16, tag="idxhi")
                nc.sync.dma_start(
                    idxhi[:], idx16_d[:, t0 * 8 + TBL * 8 : (t0 + TB) * 8]
                )
                s2bc = bpool.tile([P, P], f32, tag="s2bc")
                nc.sync.dma_start(
                    s2bc[:], s2b_d[0:1, b * P : (b + 1) * P].to_broadcast((P, P))
                )

                Y_blk = ypool.tile([P, TB * C], f32, tag="Y")
                if "gather" in skips:
                    nc.vector.memset(Y_blk[:], 0.5)
                else:
                    nc.gpsimd.dma_gather(
                        Y_blk[:, : TBL * C].rearrange("p (t c) -> p t c", c=C),
                        x_d[:],
                        idxlo[:],
                        TBL * P,
                        TBL * P,
                        C,
                        single_packet=False,
                    )
                    nc.gpsimd.dma_gather(
                        Y_blk[:, TBL * C :].rearrange("p (t c) -> p t c", c=C),
                        xhi_d[:],
                        idxhi[:],
                        TBH * P,
                        TBH * P,
                        C,
                        single_packet=False,
                    )

                sr_blk = spool.tile([P, TB], f32, tag="sr")
                sc_blk = spool.tile([P, TB], f32, tag="sc")
                if "stt" in skips:
                    nc.vector.memset(sr_blk[:], 0.1)
                    nc.vector.memset(sc_blk[:], 0.1)
                for t in range([0, TB]["stt" not in skips]):
                    scr1 = ohpool.tile([P, P], f32, tag="scr1")
                    nc.vector.scalar_tensor_tensor(
                        out=scr1[:],
                        in0=Y_blk[:, t * C : (t + 1) * C],
                        scalar=1.0,
                        in1=w1b[:],
                        op0=mybir.AluOpType.mult,
                        op1=mybir.AluOpType.mult,
                        accum_out=sr_blk[:, t : t + 1],
                    )
                    scr2 = ohpool.tile([P, P], f32, tag="scr2")
                    nc.vector.scalar_tensor_tensor(
                        out=scr2[:],
                        in0=iotaf[:],
                        scalar=colrel_blk[:, t : t + 1],
                        in1=s2bc[:],
                        op0=mybir.AluOpType.is_equal,
                        op1=mybir.AluOpType.mult,
                        accum_out=sc_blk[:, t : t + 1],
                    )

                u_blk = spool.tile([P, TB], f32, tag="u")
                nc.vector.tensor_tensor(
                    out=u_blk[:],
                    in0=sr_blk[:],
                    in1=sc_blk[:],
                    op=mybir.AluOpType.add,
                )
                th_blk = spool.tile([P, TB], f32, tag="th")
                nc.scalar.activation(
                    th_blk[:], u_blk[:], mybir.ActivationFunctionType.Tanh
                )
                p_blk = spool.tile([P, TB], f32, tag="p")
                nc.scalar.activation(
                    p_blk[:], th_blk[:], mybir.ActivationFunctionType.Exp
                )

                acc_x = bpsum.tile([P, C], f32, tag="accx")
                acc_s = bpsum.tile([P, 1], f32, tag="accs")
                # interleaving two PSUM accumulation groups wedges the PE;
                # run the acc_x group fully, then the acc_s group.
                onehots = []
                for t in range([0, TB]["mm" not in skips]):
                    onehot = ohpool.tile([P, P], f32, tag="onehot")
                    nc.vector.tensor_scalar(
                        onehot[:],
                        iotaf[:],
                        colrel_blk[:, t : t + 1],
                        p_blk[:, t : t + 1],
                        op0=mybir.AluOpType.is_equal,
                        op1=mybir.AluOpType.mult,
                    )
                    onehots.append(onehot)
                    nc.tensor.matmul(
                        out=acc_x[:],
                        lhsT=onehot[:],
                        rhs=Y_blk[:, t * C : (t + 1) * C],
                        start=(t == 0),
                        stop=(t == TB - 1),
                    )
                for t in range([0, TB]["mm" not in skips]):
                    nc.tensor.matmul(
                        out=acc_s[:],
                        lhsT=onehots[t][:],
                        rhs=ones_col[:],
                        start=(t == 0),
                        stop=(t == TB - 1),
                    )

                if "mm" in skips:
                    nc.tensor.matmul(out=acc_x[:], lhsT=iotaf[:], rhs=iotaf[:], start=True, stop=True)
                    nc.tensor.matmul(out=acc_s[:], lhsT=iotaf[:], rhs=ones_col[:], start=True, stop=True)
                segsum = spool.tile([P, 1], f32, tag="segsum")
                nc.vector.tensor_scalar(
                    segsum[:], acc_s[:], 1e-30, None, op0=mybir.AluOpType.add
                )
                inv = spool.tile([P, 1], f32, tag="inv")
                nc.vector.reciprocal(inv[:], segsum[:])
                inv9 = spool.tile([P, 1], f32, tag="inv9")
                nc.scalar.mul(inv9[:], inv[:], 1.0 - EPS)

                xblk = bpool.tile([P, C], f32, tag="xblk")
                nc.sync.dma_start(xblk[:nd, :], xloc_d[b * P : b * P + nd, :])
                o1 = bpool.tile([P, C], f32, tag="o1")
                nc.vector.tensor_scalar(
                    o1[:], acc_x[:], inv9[:], None, op0=mybir.AluOpType.mult
                )
                oblk = bpool.tile([P, C], f32, tag="oblk")
                nc.vector.scalar_tensor_tensor(
                    oblk[:nd, :],
                    xblk[:nd, :],
                    EPS,
                    o1[:nd, :],
                    op0=mybir.AluOpType.mult,
                    op1=mybir.AluOpType.add,
                )
                nc.sync.dma_start(out_d[b * P : b * P + nd, :], oblk[:nd, :])

    nc.finalize()
    return nc


_CACHE = {}


def _get_nc(TBL, TBH):
    key = (TBL, TBH)
    if key not in _CACHE:
        _CACHE[key] = _build_nc(TBL, TBH)
    return _CACHE[key]


def _make_in_maps(x, edge_index, gate_w, gate_b):
    TBL, TBH, shards = _prep_shards(edge_index)
    iotaf = np.broadcast_to(np.arange(P, dtype=np.float32)[None, :], (P, P)).copy()
    w1b = np.broadcast_to(gate_w[:C, 0][None, :], (P, C)).copy()
    xhi = np.ascontiguousarray(x[HALF:])
    in_maps = []
    for c in range(NCORES):
        xloc = np.ascontiguousarray(x[c * NLOC : (c + 1) * NLOC])
        in_maps.append(
            {
                "x": x,
                "xhi": xhi,
                "xloc": xloc,
                "xlocT": np.ascontiguousarray(xloc.T),
                "idx16": shards[c]["idx16"],
                "colrel": shards[c]["colrel_T"],
                "gate_w": gate_w,
                "gate_b": gate_b,
                "iotaf": iotaf,
                "w1b": w1b,
            }
        )
    return TBL, TBH, in_maps


def kernel(x, edge_index, gate_w, gate_b):
    from concourse.bass_utils import run_bass_kernel_spmd

    x = np.asarray(x, dtype=np.float32)
    edge_index = np.asarray(edge_index, dtype=np.int32)
    gate_w = np.asarray(gate_w, dtype=np.float32)
    gate_b = np.asarray(gate_b, dtype=np.float32)

    TBL, TBH, in_maps = _make_in_maps(x, edge_index, gate_w, gate_b)
    nc = _get_nc(TBL, TBH)

    res = run_bass_kernel_spmd(nc, in_maps, core_ids=list(range(NCORES)))
    out = np.concatenate([res.results[c]["out"] for c in range(NCORES)], axis=0)
    return out


def time_kernel(inputs, iters=32, iters_lo=2, reps=4):
    """Estimate per-execution HW time: async-dispatch M executions of one jitted
    single-exec program (device executions serialize per core); per-exec time =
    (T(M_hi) - T(M_lo)) / (M_hi - M_lo), min over reps."""
    import time as _time

    import jax
    import concourse.mybir as mybir
    from concourse import bass2jax as b2j

    x = np.asarray(inputs["x"], dtype=np.float32)
    edge_index = np.asarray(inputs["edge_index"], dtype=np.int32)
    gate_w = np.asarray(inputs["gate_w"], dtype=np.float32)
    gate_b = np.asarray(inputs["gate_b"], dtype=np.float32)

    TBL, TBH, in_maps = _make_in_maps(x, edge_index, gate_w, gate_b)
    nc = _get_nc(TBL, TBH)
    b2j.install_neuronx_cc_hook()

    partition_name = nc.partition_id_tensor.name if nc.partition_id_tensor else None
    in_names, out_names, out_avals, zero_outs = [], [], [], []
    for alloc in nc.m.functions[0].allocations:
        if not isinstance(alloc, mybir.MemoryLocationSet):
            continue
        name = alloc.memorylocations[0].name
        if alloc.kind == "ExternalInput":
            if name != partition_name:
                in_names.append(name)
        elif alloc.kind == "ExternalOutput":
            shape = tuple(alloc.tensor_shape)
            dtype = mybir.dt.np(alloc.dtype)
            out_names.append(name)
            out_avals.append(jax.core.ShapedArray(shape, dtype))
            zero_outs.append(np.zeros(shape, dtype))
    n_params = len(in_names)
    all_in_names = in_names + out_names

    def _body(*args):
        operands = list(args)
        if partition_name is not None:
            operands.append(b2j.partition_id_tensor())
        return tuple(
            b2j._bass_exec_p.bind(
                *operands,
                out_avals=tuple(out_avals),
                in_names=tuple(
                    all_in_names + ([partition_name] if partition_name else [])
                ),
                out_names=tuple(out_names),
                lowering_input_output_aliases=(),
                sim_require_finite=True,
                sim_require_nnan=True,
                nc=nc,
            )
        )

    devices = jax.devices()[:NCORES]
    mesh = b2j.Mesh(np.asarray(devices), ("core",))
    in_specs = (b2j.PartitionSpec("core",),) * (n_params + len(out_names))
    out_specs = (b2j.PartitionSpec("core",),) * len(out_names)
    fn = jax.jit(
        b2j.shard_map(
            _body, mesh=mesh, in_specs=in_specs, out_specs=out_specs, check_rep=False
        ),
        keep_unused=True,
    )

    per_core = [[np.asarray(m[name]) for name in in_names] for m in in_maps]
    concat_in = [
        np.concatenate([per_core[c][i] for c in range(NCORES)], axis=0)
        for i in range(n_params)
    ]
    concat_zeros = [
        np.zeros((NCORES * z.shape[0], *z.shape[1:]), z.dtype) for z in zero_outs
    ]

    from jax.sharding import NamedSharding

    sh = NamedSharding(mesh, b2j.PartitionSpec("core"))
    dev_in = [jax.device_put(a, sh) for a in concat_in]
    dev_zero = [jax.device_put(a, sh) for a in concat_zeros]

    jax.block_until_ready(fn(*dev_in, *dev_zero))
    jax.block_until_ready(fn(*dev_in, *dev_zero))

    best = None
    for _ in range(reps):
        t0 = _time.perf_counter()
        rs = [fn(*dev_in, *dev_zero) for _ in range(iters)]
        jax.block_until_ready(rs)
        t_hi = _time.perf_counter() - t0
        del rs
        t0 = _time.perf_counter()
        rs = [fn(*dev_in, *dev_zero) for _ in range(iters_lo)]
        jax.block_until_ready(rs)
        t_lo = _time.perf_counter() - t0
        del rs
        per_exec = (t_hi - t_lo) / (iters - iters_lo)
        print(
            f"  t({iters})={t_hi*1e3:.2f}ms t({iters_lo})={t_lo*1e3:.2f}ms "
            f"per_exec={per_exec*1e6:.1f}us"
        )
        if best is None or per_exec < best:
            best = per_exec
    return best * 1e9
